# revision 14
# baseline (speedup 1.0000x reference)
"""AttentionSequencePoolingLayer (DIN-style) Trainium2 Bass kernel.

Math (per batch b, position t):
  att_in = [q, k, q-k, q*k] @ W1 + b1
         = k @ A + (q*k) @ P + (q @ (W1q+W1d) + b1)     [algebraic refactor]
    where W1 = [W1q; W1k; W1d; W1p], A = W1k - W1d, P = W1p.
  h1 = sigmoid(...); h2 = sigmoid(h1 @ W2 + b2); score = h2 @ W3 (+ b3,
  dropped: softmax is shift-invariant).  Masked positions get -80 (exp ~ 0).
  out[b] = softmax(score) @ keys[b].

Layout strategy (per core, 512 batches, all on-chip data bf16):
  - batches sorted by length desc (host); per-slot compute caps = max len
    across cores (one SPMD program serves all 8 cores).  All MLP matmuls,
    sigmoids and score copies are truncated to the cap.
  - kT2 HBM layout [128, BC*T/2]: partition p=(hi,e) holds keys feature e
    for 16-batch half hi of each 32-batch group; every keys DMA is a plain
    [128, N] copy whose descriptors spread across all 16 DMA engines (a
    2-outer-dim AP confines a DMA to 2 engines).
  - MLP runs in transposed layout (features on partitions, (b,t) on free
    dim), batches PACKED into <=512-column PSUM tiles: nb consecutive
    batches share one tile at the pack's max cap, so sigmoid/copies are
    dense (no per-batch slicing) and per-matmul dispatch is amortized.
  - per-batch layer-1 bias (aT = q@(W1q+W1d)+b1, one matmul per supertile
    at setup) enters through a K=128 identity-selector matmul.
  - scores land in [2, nb*cg] PSUM, are copied (DVE) into a [2, 32*200]
    bf16 staging row at 200/batch stride, then 2 strided SBUF->SBUF DMAs
    per 32 batches relayout to a [128b, 200t] strip for softmax (ACT exp
    with fused f32 sum).
  - weighted sum from natural-layout keys (knat, bf16, truncated to the
    supertile cap): multiply on GPSIMD (idle engine), segmented t-reduce
    on DVE (f32 accum), then combine + 1/sum normalize.
  - length-0 batches (softmax over all-masked = uniform mean of keys) are
    computed host-side and overwrite those rows; the device path may
    produce garbage for them.

Compiler workaround: this container's walrus rejects instructions with
more than one semaphore wait; _legalize_waits() rewrites the BIR so every
excess wait rides its own same-engine EventSemaphore.
"""

import json
import sys

import numpy as np

try:
    import concourse.bass as bass
except ImportError:
    sys.path.insert(0, "/opt/trn_rl_repo")
    import concourse.bass as bass
import concourse.mybir as mybir
import concourse.tile as tile
from concourse.bass_utils import run_bass_kernel_spmd

E = 64
T = 200
H1, H2 = 80, 40
NCORES = 8
BC = 4096 // NCORES          # batches per core
NSUPER = BC // 128           # supertiles of 128 batches
NGRP = BC // 32              # 32-batch keys groups (one kdual DMA each)
MASK_NEG = -80.0
PSUM_COLS = 512

F32 = mybir.dt.float32
BF16 = mybir.dt.bfloat16


def _bcast(ap2d, c0, nb, nt):
    """From [P, C] SBUF ap: [P, nb, nt] AP broadcasting col c over nt."""
    base = ap2d[:, c0 : c0 + nb]
    return bass.AP(
        tensor=base.tensor,
        offset=base.offset,
        ap=[base.ap[0], base.ap[1], [0, nt]],
    )


def _make_packs(caps):
    """Per supertile: list of (slot, nb, cg) packs.  Slots are sorted by
    cap desc; packs stay inside 16-slot halves so each maps to one
    partition half of its 32-batch group."""
    packs = []
    for s in range(NSUPER):
        ps = []
        b = 0
        while b < 128:
            cg = caps[s * 128 + b]
            nb = min(PSUM_COLS // cg, 16 - b % 16)
            ps.append((b, nb, cg))
            b += nb
        packs.append(ps)
    return packs


def build_nc(packs, tcs_list):
    nc = bass.Bass("TRN2")

    kT2 = nc.dram_tensor("kT2", [E, BC * T], BF16, kind="ExternalInput")
    qkT2 = nc.dram_tensor("qkT2", [E, BC * T], BF16, kind="ExternalInput")
    knat = nc.dram_tensor("knat", [BC, T * E], BF16, kind="ExternalInput")
    wall = nc.dram_tensor("wall", [128, 1804 + 16 * T], BF16, kind="ExternalInput")
    out = nc.dram_tensor("out", [BC, E], F32, kind="ExternalOutput")

    with tile.TileContext(nc) as tc:
        with (
            tc.tile_pool(name="consts", bufs=1) as consts,
            tc.tile_pool(name="ktp", bufs=4) as ktp,
            tc.tile_pool(name="qkp", bufs=4) as qkp,
            tc.tile_pool(name="lgp", bufs=4) as lgp,
            tc.tile_pool(name="h1p", bufs=3) as h1p,
            tc.tile_pool(name="h2p", bufs=3) as h2p,
            tc.tile_pool(name="scp", bufs=2) as scp,
            tc.tile_pool(name="stripp", bufs=2) as stripp,
            tc.tile_pool(name="softp", bufs=2) as softp,
            tc.tile_pool(name="knp", bufs=2) as knp,
            tc.tile_pool(name="outp", bufs=2) as outp,
            tc.tile_pool(name="ps1", bufs=3, space="PSUM") as ps1,
            tc.tile_pool(name="ps2", bufs=2, space="PSUM") as ps2,
            tc.tile_pool(name="ps3", bufs=2, space="PSUM") as ps3,
            tc.tile_pool(name="psg", bufs=1, space="PSUM") as psg,
        ):
            # ---- constants (one DMA for all weights + qT/qsh + mask) ----
            sb_wall0 = consts.tile([128, 1804 + 16 * T], BF16)
            nc.sync.dma_start(out=sb_wall0, in_=wall[:, :])
            # copy through DVE so every consumer waits on an engine sem
            sb_wall = consts.tile([128, 1804 + 16 * T], BF16)
            nc.vector.tensor_copy(out=sb_wall, in_=sb_wall0)
            sb_A = sb_wall[0:E, 0:H1]
            sb_P = sb_wall[0:E, 80:160]
            sb_Wqd = sb_wall[0:E, 160:240]
            sb_W2 = sb_wall[0:H1, 240:280]
            sb_W3r = sb_wall[0:H2, 280:282]
            sb_b2 = sb_wall[0:H2, 282:283]
            sb_b1rep16 = sb_wall[E : E + 16, 284:364]
            sb_mask = sb_wall[:, 492 : 492 + NSUPER * T]
            sb_qT = sb_wall[0:E, 1292 : 1292 + BC]          # [64, BC]
            sb_oh = sb_wall[E : E + 16, 1804 : 1804 + 16 * T]

            # seed the rotating pool slots: keys tiles get the constant
            # one-hot selector rows (64:80), lhsT tiles get A (rows 0:64);
            # later writers only touch the complementary partition rows, so
            # these persist across slot reuse
            for _ in range(4):
                kd0 = ktp.tile([E + 16, 16 * T], BF16, tag="kd")
                nc.vector.tensor_copy(out=kd0[E : E + 16, :], in_=sb_oh)
                lg0 = lgp.tile([H1, H1], BF16, tag="lg")
                nc.vector.tensor_copy(out=lg0[0:E, :], in_=sb_A)

            # staging slots hold stale data beyond each batch's cap; first
            # use must be finite (later reuse leaves bounded old scores)
            for _ in range(2):
                sc_t = scp.tile([2, 32 * T], BF16, tag="sc")
                nc.vector.memset(sc_t[:, :], MASK_NEG)

            # ---- 16-batch group prefetch: keys DMA (rows 0:64), q*k on
            # GPSIMD, and the merged lhsT [A; aT16] (bias rows via a tiny
            # base-64 matmul + DVE add; one-hot rhs rows make the bias land
            # per batch, replacing the old K=128 selector matmul) ----
            kduals = {}

            def prefetch(g):
                kd = ktp.tile([E + 16, 16 * T], BF16, tag="kd")
                nc.sync.dma_start(
                    out=kd[0:E, :], in_=kT2[:, g * 16 * T : (g + 1) * 16 * T]
                )
                qk = qkp.tile([E, 16 * T], BF16, tag="qk")
                nc.sync.dma_start(
                    out=qk, in_=qkT2[:, g * 16 * T : (g + 1) * 16 * T]
                )
                aT_ps = psg.tile([128, H1], F32, tag="psg")
                nc.tensor.matmul(
                    aT_ps[E : E + 16, :],
                    sb_qT[:, g * 16 : (g + 1) * 16],
                    sb_Wqd,
                    start=True,
                    stop=True,
                )
                lg = lgp.tile([H1, H1], BF16, tag="lg")
                nc.vector.tensor_tensor(
                    out=lg[E : E + 16, :],
                    in0=aT_ps[E : E + 16, :],
                    in1=sb_b1rep16,
                    op=mybir.AluOpType.add,
                )
                kduals[g] = (kd, qk, lg)

            prefetch(0)
            prefetch(1)

            for s in range(NSUPER):
                strip = stripp.tile([128, T], BF16)
                plist = packs[s]
                n = len(plist)
                blk_last = {}          # g4 -> index of last pack in block
                for i, p in enumerate(plist):
                    blk_last[p[0] // 32] = i
                st_ = {}               # pack idx -> per-stage artifacts
                sc_blk = {}            # g4 -> staging tile

                def stA(i):
                    gb, nb, cg = plist[i]
                    g4 = gb // 32
                    if gb % 16 == 0:   # first pack of a 16-batch group
                        nxt = s * 8 + gb // 16 + 2
                        if nxt < NSUPER * 8:
                            prefetch(nxt)
                    if gb % 32 == 0:   # first pack of a staging block
                        sc_t = scp.tile([2, 32 * T], BF16, tag="sc")
                        # tiny touch so the slot-reuse DMA waits land here
                        nc.vector.memset(sc_t[0:2, 0:1], MASK_NEG)
                        sc_blk[g4] = sc_t
                    kd, qk, lg = kduals[s * 8 + gb // 16]
                    j0 = gb % 16
                    ncol = nb * cg
                    kv = kd.rearrange("p (b t) -> p b t", t=T)
                    qv = qk.rearrange("p (b t) -> p b t", t=T)
                    p1 = ps1.tile([H1, PSUM_COLS], F32)
                    nc.tensor.matmul(
                        p1[:, 0:ncol], lg, kv[:, j0 : j0 + nb, 0:cg],
                        start=True, stop=False,
                    )
                    nc.tensor.matmul(
                        p1[:, 0:ncol], sb_P, qv[:, j0 : j0 + nb, 0:cg],
                        start=False, stop=True,
                    )
                    st_[i] = {"p1": p1}

                def stB(i):
                    gb, nb, cg = plist[i]
                    ncol = nb * cg
                    h1_t = h1p.tile([H1, PSUM_COLS], BF16, tag="h1_t")
                    nc.scalar.activation(
                        out=h1_t[:, 0:ncol],
                        in_=st_[i]["p1"][:, 0:ncol],
                        func=mybir.ActivationFunctionType.Sigmoid,
                    )
                    st_[i]["h1"] = h1_t

                def stC(i):
                    gb, nb, cg = plist[i]
                    ncol = nb * cg
                    p2 = ps2.tile([H2, PSUM_COLS], F32)
                    nc.tensor.matmul(
                        p2[:, 0:ncol], sb_W2, st_[i]["h1"][:, 0:ncol],
                        start=True, stop=True,
                    )
                    st_[i]["p2"] = p2

                def stD(i):
                    gb, nb, cg = plist[i]
                    ncol = nb * cg
                    h2_t = h2p.tile([H2, PSUM_COLS], BF16, tag="h2_t")
                    nc.scalar.activation(
                        out=h2_t[:, 0:ncol],
                        in_=st_[i]["p2"][:, 0:ncol],
                        func=mybir.ActivationFunctionType.Sigmoid,
                        bias=sb_b2[:, 0:1],
                    )
                    st_[i]["h2"] = h2_t

                def stE(i):
                    gb, nb, cg = plist[i]
                    ncol = nb * cg
                    p3 = ps3.tile([2, PSUM_COLS], F32)
                    nc.tensor.matmul(
                        p3[:, 0:ncol], sb_W3r, st_[i]["h2"][:, 0:ncol],
                        start=True, stop=True,
                    )
                    st_[i]["p3"] = p3

                def stF(i):
                    gb, nb, cg = plist[i]
                    g4 = gb // 32
                    sa = sc_blk[g4][:]
                    p3t = st_[i]["p3"][:]
                    nc.vector.tensor_copy(
                        out=bass.AP(
                            tensor=sa.tensor,
                            offset=sa.offset + (gb - g4 * 32) * T,
                            ap=[sa.ap[0], [T, nb], [1, cg]],
                        ),
                        in_=bass.AP(
                            tensor=p3t.tensor,
                            offset=p3t.offset,
                            ap=[p3t.ap[0], [cg, nb], [1, cg]],
                        ),
                    )
                    st_[i] = None
                    if blk_last[g4] == i:
                        # relayout 32 batches of scores into the strip
                        st = strip[:]
                        for r in range(2):
                            nc.sync.dma_start(
                                out=bass.AP(
                                    tensor=st.tensor,
                                    offset=st.offset
                                    + (g4 * 32 + r) * st.ap[0][0],
                                    ap=[[2 * st.ap[0][0], 16], [1, T]],
                                ),
                                in_=bass.AP(
                                    tensor=sa.tensor,
                                    offset=sa.offset + r * sa.ap[0][0] + r * T,
                                    ap=[[sa.ap[0][0], 1], [2 * T, 16], [1, T]],
                                ),
                            )
                        del sc_blk[g4]

                # 2-stage software pipeline: PE gets A(i), C(i-1), E(i-2);
                # ACT gets B(i), D(i-1); DVE gets F(i-2).  Engine queues
                # are in-order, so the skew keeps every queue head's
                # dependencies already satisfied.
                for i in range(n + 2):
                    if i < n:
                        stA(i)
                    if i >= 1 and i - 1 < n:
                        stC(i - 1)
                    if i >= 2:
                        stE(i - 2)
                    if i < n:
                        stB(i)
                    if i >= 1 and i - 1 < n:
                        stD(i - 1)
                    if i >= 2:
                        stF(i - 2)

                # ---- softmax over t for 128 batches ----
                nc.vector.tensor_tensor(
                    out=strip,
                    in0=strip,
                    in1=sb_mask[:, s * T : (s + 1) * T],
                    op=mybir.AluOpType.add,
                )
                ew = softp.tile([128, T], BF16)
                esum = softp.tile([128, 1], F32)
                nc.scalar.activation(
                    out=ew,
                    in_=strip,
                    func=mybir.ActivationFunctionType.Exp,
                    accum_out=esum,
                )
                rsum = softp.tile([128, 1], F32)
                nc.vector.reciprocal(out=rsum, in_=esum)

                # ---- weighted sum: out[b, e] = sum_t w[b,t] * keys[b,t,e]
                TH = T // 2
                tcs = tcs_list[s]
                o_h = []
                for h in range(2):
                    tc_h = min(TH, max(0, tcs - h * TH))
                    if tc_h == 0:
                        continue
                    kn_t = knp.tile([128, TH * E], BF16, tag="kn_t")
                    nc.sync.dma_start(
                        out=kn_t[:, 0 : tc_h * E],
                        in_=knat[
                            s * 128 : (s + 1) * 128,
                            h * TH * E : (h * TH + tc_h) * E,
                        ],
                    )
                    ewap = ew[:]
                    w_b = bass.AP(
                        tensor=ewap.tensor,
                        offset=ewap.offset + h * TH,
                        ap=[ewap.ap[0], [1, tc_h], [0, E]],
                    )
                    kview = kn_t.rearrange("b (t e) -> b t e", e=E)[:, 0:tc_h, :]
                    nc.gpsimd.tensor_tensor(
                        out=kview, in0=kview, in1=w_b, op=mybir.AluOpType.mult
                    )
                    o_t = outp.tile([128, E], F32, tag=f"oh{h}")
                    nc.vector.tensor_reduce(
                        out=o_t,
                        in_=bass.AP(
                            tensor=kn_t.tensor,
                            offset=kn_t[:].offset,
                            ap=[kn_t[:].ap[0], [1, E], [E, tc_h]],
                        ),
                        axis=mybir.AxisListType.X,
                        op=mybir.AluOpType.add,
                    )
                    o_h.append(o_t)
                if len(o_h) == 2:
                    o_s = outp.tile([128, E], F32, tag="os")
                    nc.vector.tensor_add(out=o_s, in0=o_h[0], in1=o_h[1])
                else:
                    o_s = o_h[0]
                o_f = outp.tile([128, E], F32, tag="of")
                rs = rsum[:]
                nc.vector.tensor_tensor(
                    out=o_f,
                    in0=o_s,
                    in1=bass.AP(tensor=rs.tensor, offset=rs.offset,
                                ap=[rs.ap[0], [0, E]]),
                    op=mybir.AluOpType.mult,
                )
                nc.sync.dma_start(out=out[s * 128 : (s + 1) * 128, :], in_=o_f)

    return nc


_SEQ_OK = {"EventSemaphore", "ISA", "RegisterMove", "RegisterAluOp"}


def _legalize_waits(bir_bytes):
    """This container's walrus rejects compute instructions that carry a
    DMA-semaphore wait alongside any other wait ("Too many sync wait
    commands").  Move every DMA-sem wait of a multi-wait compute
    instruction onto its own same-engine EventSemaphore (pure sequencer
    wait) inserted right before it -- semantics are identical, the
    sequencer simply performs the waits one instruction earlier."""
    d = json.loads(bir_bytes)
    for fn in d["functions"]:
        for bb in fn["blocks"]:
            out = []
            for ins in bb["instructions"]:
                si = ins.get("sync_info")
                waits = (si or {}).get("on_wait") or []
                if si and len(waits) >= 2 and ins.get("opcode") not in _SEQ_OK:
                    eng = [
                        w
                        for w in waits
                        if not str(w.get("ant_name", "")).startswith("DMA")
                    ]
                    kept = eng[-1] if eng else waits[-1]
                    moved = [w for w in waits if w is not kept]
                    for k, w in enumerate(moved):
                        out.append(
                            {
                                "name": f"{ins['name']}_lw{k}",
                                "opcode": "EventSemaphore",
                                "engine": ins["engine"],
                                "debug": ins.get("debug", 0),
                                "ins": [],
                                "outs": [],
                                "sync_info": {
                                    "on_wait": [w],
                                    "on_update": [],
                                },
                            }
                        )
                    si["on_wait"] = [kept]
                out.append(ins)
            bb["instructions"] = out
    return json.dumps(d).encode()


import ml_dtypes

BF16_NP = np.dtype(ml_dtypes.bfloat16)


def _prep_wall(W1, b1, W2, b2, W3):
    W1 = np.asarray(W1, np.float32)
    W1q, W1k, W1d, W1p = W1[0:64], W1[64:128], W1[128:192], W1[192:256]
    wall = np.zeros((128, 1492), np.float32)
    wall[0:64, 0:80] = W1k - W1d          # A
    wall[0:64, 80:160] = W1p              # P
    wall[0:64, 160:240] = W1q + W1d       # Wqd
    wall[64:128, 0:80] = wall[0:64, 0:80]
    wall[64:128, 80:160] = wall[0:64, 80:160]
    wall[0:80, 240:280] = np.asarray(W2, np.float32)
    wall[0:40, 280:282] = np.repeat(np.asarray(W3, np.float32), 2, axis=1)
    wall[0:40, 282] = np.asarray(b2, np.float32)
    wall[:, 284:364] = np.asarray(b1, np.float32)[None, :]
    wall[:, 364:492] = np.eye(128, dtype=np.float32)
    return wall


def kernel(query, keys, keys_length, W1, b1, W2, b2, W3, b3, _trace=False):
    query = np.asarray(query, np.float32)
    keys = np.asarray(keys, np.float32)
    lens = np.asarray(keys_length).reshape(4096, 1)

    wall_w = _prep_wall(W1, b1, W2, b2, W3)

    # sort each core's batches by length (desc); compute caps are the
    # per-slot max across cores, so one SPMD program serves all 8 cores
    orders = [
        np.argsort(-lens[c * BC : (c + 1) * BC, 0], kind="stable")
        for c in range(NCORES)
    ]
    sorted_lens = np.stack(
        [lens[c * BC : (c + 1) * BC, 0][orders[c]] for c in range(NCORES)]
    )
    caps = np.clip(
        (np.max(sorted_lens, axis=0).astype(np.int64) + 7) // 8 * 8, 8, T
    )
    caps = [int(x) for x in caps]
    # weighted-sum truncation: supertile max cap (len-0 batches are
    # handled host-side below)
    tcs_list = [int(caps[s * 128]) for s in range(NSUPER)]
    packs = _make_packs(caps)
    nc = build_nc(packs, tcs_list)
    patched = _legalize_waits(nc.to_json_bytes())
    nc.to_json_bytes = lambda: patched

    in_maps = []
    for c in range(NCORES):
        od = orders[c]
        kc = keys[c * BC : (c + 1) * BC][od]                  # [BC, T, E]
        qc = query[c * BC : (c + 1) * BC, 0, :][od]           # [BC, E]
        lc = lens[c * BC : (c + 1) * BC, 0][od].astype(np.int64)
        tt = np.arange(T)[None, :]
        mc = np.where(tt < lc[:, None], 0.0, MASK_NEG).astype(np.float32)
        # [BC, T] -> [128, NSUPER*T]: column-block s holds supertile s
        mc = np.ascontiguousarray(
            mc.reshape(NSUPER, 128, T).transpose(1, 0, 2).reshape(128, NSUPER * T)
        )
        kcb = kc.astype(BF16_NP)
        # kT2 [64, BC*T]: plain e-on-partition transposed keys, batch-major
        kT2 = np.ascontiguousarray(
            kcb.transpose(2, 0, 1).reshape(E, BC * T)
        )
        qkT2 = np.ascontiguousarray(
            (qc[:, None, :] * kc).astype(BF16_NP)
            .transpose(2, 0, 1).reshape(E, BC * T)
        )
        qcT = qc.T                               # [64, BC]
        wqc = np.zeros((128, 1804 + 16 * T), np.float32)
        wqc[:, 0:492] = wall_w[:, 0:492]
        wqc[:, 492 : 492 + NSUPER * T] = mc
        wqc[0:E, 1292 : 1292 + BC] = qcT
        # one-hot selector rows (partitions 64:80): row j = 1 on batch j's
        # 200 t-columns of a 16-batch keys tile
        for j in range(16):
            wqc[E + j, 1804 + j * T : 1804 + (j + 1) * T] = 1.0
        in_maps.append(
            {
                "kT2": kT2,
                "qkT2": qkT2,
                "knat": np.ascontiguousarray(kcb.reshape(BC, T * E)),
                "wall": np.ascontiguousarray(wqc.astype(BF16_NP)),
            }
        )

    res = run_bass_kernel_spmd(nc, in_maps, core_ids=list(range(NCORES)),
                               trace=_trace)
    outs = []
    for c in range(NCORES):
        blk = np.empty((BC, E), np.float32)
        blk[orders[c]] = res.results[c]["out"]
        # length-0 batches: reference softmax is uniform -> mean of keys
        lc = lens[c * BC : (c + 1) * BC, 0]
        z = np.nonzero(lc == 0)[0]
        if len(z):
            blk[z] = keys[c * BC : (c + 1) * BC][z].mean(axis=1)
        outs.append(blk)
    full = np.concatenate(outs, axis=0)[:, None, :]
    if _trace:
        kernel._last_exec_ns = res.exec_time_ns
        kernel._last_results = res
    return full.astype(np.float32)


# revision 15
# speedup vs baseline: 1.0281x; 1.0281x over previous
"""AttentionSequencePoolingLayer (DIN-style) Trainium2 Bass kernel.

Math (per batch b, position t):
  att_in = [q, k, q-k, q*k] @ W1 + b1
         = k @ A + (q*k) @ P + (q @ (W1q+W1d) + b1)     [algebraic refactor]
    where W1 = [W1q; W1k; W1d; W1p], A = W1k - W1d, P = W1p.
  h1 = sigmoid(...); h2 = sigmoid(h1 @ W2 + b2); score = h2 @ W3 (+ b3,
  dropped: softmax is shift-invariant).  Masked positions get -80 (exp ~ 0).
  out[b] = softmax(score) @ keys[b].

Layout strategy (per core, 512 batches, all on-chip data bf16):
  - batches sorted by length desc (host); per-slot compute caps = max len
    across cores (one SPMD program serves all 8 cores).  All MLP matmuls,
    sigmoids and score copies are truncated to the cap.
  - kT2 HBM layout [128, BC*T/2]: partition p=(hi,e) holds keys feature e
    for 16-batch half hi of each 32-batch group; every keys DMA is a plain
    [128, N] copy whose descriptors spread across all 16 DMA engines (a
    2-outer-dim AP confines a DMA to 2 engines).
  - MLP runs in transposed layout (features on partitions, (b,t) on free
    dim), batches PACKED into <=512-column PSUM tiles: nb consecutive
    batches share one tile at the pack's max cap, so sigmoid/copies are
    dense (no per-batch slicing) and per-matmul dispatch is amortized.
  - per-batch layer-1 bias (aT = q@(W1q+W1d)+b1, one matmul per supertile
    at setup) enters through a K=128 identity-selector matmul.
  - scores land in [2, nb*cg] PSUM, are copied (DVE) into a [2, 32*200]
    bf16 staging row at 200/batch stride, then 2 strided SBUF->SBUF DMAs
    per 32 batches relayout to a [128b, 200t] strip for softmax (ACT exp
    with fused f32 sum).
  - weighted sum from natural-layout keys (knat, bf16, truncated to the
    supertile cap): multiply on GPSIMD (idle engine), segmented t-reduce
    on DVE (f32 accum), then combine + 1/sum normalize.
  - length-0 batches (softmax over all-masked = uniform mean of keys) are
    computed host-side and overwrite those rows; the device path may
    produce garbage for them.

Compiler workaround: this container's walrus rejects instructions with
more than one semaphore wait; _legalize_waits() rewrites the BIR so every
excess wait rides its own same-engine EventSemaphore.
"""

import json
import sys

import numpy as np

try:
    import concourse.bass as bass
except ImportError:
    sys.path.insert(0, "/opt/trn_rl_repo")
    import concourse.bass as bass
import concourse.mybir as mybir
import concourse.tile as tile
from concourse.bass_utils import run_bass_kernel_spmd

E = 64
T = 200
H1, H2 = 80, 40
NCORES = 8
BC = 4096 // NCORES          # batches per core
NSUPER = BC // 128           # supertiles of 128 batches
NGRP = BC // 32              # 32-batch keys groups (one kdual DMA each)
MASK_NEG = -80.0
PSUM_COLS = 512

F32 = mybir.dt.float32
BF16 = mybir.dt.bfloat16


def _bcast(ap2d, c0, nb, nt):
    """From [P, C] SBUF ap: [P, nb, nt] AP broadcasting col c over nt."""
    base = ap2d[:, c0 : c0 + nb]
    return bass.AP(
        tensor=base.tensor,
        offset=base.offset,
        ap=[base.ap[0], base.ap[1], [0, nt]],
    )


def _make_packs(caps):
    """Per supertile: list of (slot, nb, cg) packs.  Slots are sorted by
    cap desc; packs stay inside 16-slot halves so each maps to one
    partition half of its 32-batch group."""
    packs = []
    for s in range(NSUPER):
        ps = []
        b = 0
        while b < 128:
            cg = caps[s * 128 + b]
            nb = min(PSUM_COLS // cg, 16 - b % 16)
            ps.append((b, nb, cg))
            b += nb
        packs.append(ps)
    return packs


def build_nc(packs, tcs_list):
    nc = bass.Bass("TRN2")

    kT2 = nc.dram_tensor("kT2", [E, BC * T], BF16, kind="ExternalInput")
    knat = nc.dram_tensor("knat", [BC, T * E], BF16, kind="ExternalInput")
    wall = nc.dram_tensor("wall", [128, 1804 + 16 * T], BF16, kind="ExternalInput")
    out = nc.dram_tensor("out", [BC, E], F32, kind="ExternalOutput")

    with tile.TileContext(nc) as tc:
        with (
            tc.tile_pool(name="consts", bufs=1) as consts,
            tc.tile_pool(name="ktp", bufs=4) as ktp,
            tc.tile_pool(name="qkp", bufs=4) as qkp,
            tc.tile_pool(name="lgp", bufs=4) as lgp,
            tc.tile_pool(name="h1p", bufs=3) as h1p,
            tc.tile_pool(name="h2p", bufs=3) as h2p,
            tc.tile_pool(name="scp", bufs=2) as scp,
            tc.tile_pool(name="stripp", bufs=2) as stripp,
            tc.tile_pool(name="softp", bufs=2) as softp,
            tc.tile_pool(name="knp", bufs=2) as knp,
            tc.tile_pool(name="outp", bufs=2) as outp,
            tc.tile_pool(name="ps1", bufs=3, space="PSUM") as ps1,
            tc.tile_pool(name="ps2", bufs=2, space="PSUM") as ps2,
            tc.tile_pool(name="ps3", bufs=2, space="PSUM") as ps3,
            tc.tile_pool(name="psg", bufs=1, space="PSUM") as psg,
        ):
            # ---- constants (one DMA for all weights + qT/qsh + mask) ----
            sb_wall0 = consts.tile([128, 1804 + 16 * T], BF16)
            nc.sync.dma_start(out=sb_wall0, in_=wall[:, :])
            # copy through DVE so every consumer waits on an engine sem
            sb_wall = consts.tile([128, 1804 + 16 * T], BF16)
            nc.vector.tensor_copy(out=sb_wall, in_=sb_wall0)
            sb_A = sb_wall[0:E, 0:H1]
            sb_P = sb_wall[0:E, 80:160]
            sb_Wqd = sb_wall[0:E, 160:240]
            sb_W2 = sb_wall[0:H1, 240:280]
            sb_W3r = sb_wall[0:H2, 280:282]
            sb_b2 = sb_wall[0:H2, 282:283]
            sb_b1rep16 = sb_wall[E : E + 16, 284:364]
            sb_mask = sb_wall[:, 492 : 492 + NSUPER * T]
            sb_qT = sb_wall[0:E, 1292 : 1292 + BC]          # [64, BC]
            sb_oh = sb_wall[E : E + 16, 1804 : 1804 + 16 * T]

            # seed the rotating pool slots: keys tiles get the constant
            # one-hot selector rows (64:80), lhsT tiles get A (rows 0:64);
            # later writers only touch the complementary partition rows, so
            # these persist across slot reuse
            for _ in range(4):
                kd0 = ktp.tile([E + 16, 16 * T], BF16, tag="kd")
                nc.vector.tensor_copy(out=kd0[E : E + 16, :], in_=sb_oh)
                lg0 = lgp.tile([H1, H1], BF16, tag="lg")
                nc.vector.tensor_copy(out=lg0[0:E, :], in_=sb_A)

            # staging slots hold stale data beyond each batch's cap; first
            # use must be finite (later reuse leaves bounded old scores)
            for _ in range(2):
                sc_t = scp.tile([2, 32 * T], BF16, tag="sc")
                nc.vector.memset(sc_t[:, :], MASK_NEG)

            # ---- 16-batch group prefetch: keys DMA (rows 0:64), q*k on
            # GPSIMD, and the merged lhsT [A; aT16] (bias rows via a tiny
            # base-64 matmul + DVE add; one-hot rhs rows make the bias land
            # per batch, replacing the old K=128 selector matmul) ----
            kduals = {}

            def prefetch(g):
                s_, gl_ = g // 8, g % 8
                kd = ktp.tile([E + 16, 16 * T], BF16, tag="kd")
                nc.sync.dma_start(
                    out=kd[0:E, :], in_=kT2[:, g * 16 * T : (g + 1) * 16 * T]
                )
                cgg = packs[s_][
                    [i for i, p in enumerate(packs[s_]) if p[0] == gl_ * 16][0]
                ][2]
                qk = qkp.tile([E, 16 * T], BF16, tag="qk")
                nc.gpsimd.tensor_tensor(
                    out=qk.rearrange("p (b t) -> p b t", t=T)[:, :, 0:cgg],
                    in0=kd[0:E, :].rearrange("p (b t) -> p b t", t=T)[
                        :, :, 0:cgg
                    ],
                    in1=_bcast(sb_qT, g * 16, 16, cgg),
                    op=mybir.AluOpType.mult,
                )
                aT_ps = psg.tile([128, H1], F32, tag="psg")
                nc.tensor.matmul(
                    aT_ps[E : E + 16, :],
                    sb_qT[:, g * 16 : (g + 1) * 16],
                    sb_Wqd,
                    start=True,
                    stop=True,
                )
                lg = lgp.tile([H1, H1], BF16, tag="lg")
                nc.vector.tensor_tensor(
                    out=lg[E : E + 16, :],
                    in0=aT_ps[E : E + 16, :],
                    in1=sb_b1rep16,
                    op=mybir.AluOpType.add,
                )
                kduals[g] = (kd, qk, lg)

            prefetch(0)
            prefetch(1)

            for s in range(NSUPER):
                strip = stripp.tile([128, T], BF16)
                plist = packs[s]
                n = len(plist)
                blk_last = {}          # g4 -> index of last pack in block
                for i, p in enumerate(plist):
                    blk_last[p[0] // 32] = i
                st_ = {}               # pack idx -> per-stage artifacts
                sc_blk = {}            # g4 -> staging tile

                def stA(i):
                    gb, nb, cg = plist[i]
                    g4 = gb // 32
                    if gb % 16 == 0:   # first pack of a 16-batch group
                        nxt = s * 8 + gb // 16 + 2
                        if nxt < NSUPER * 8:
                            prefetch(nxt)
                    if gb % 32 == 0:   # first pack of a staging block
                        sc_t = scp.tile([2, 32 * T], BF16, tag="sc")
                        # tiny touch so the slot-reuse DMA waits land here
                        nc.vector.memset(sc_t[0:2, 0:1], MASK_NEG)
                        sc_blk[g4] = sc_t
                    kd, qk, lg = kduals[s * 8 + gb // 16]
                    j0 = gb % 16
                    ncol = nb * cg
                    kv = kd.rearrange("p (b t) -> p b t", t=T)
                    qv = qk.rearrange("p (b t) -> p b t", t=T)
                    p1 = ps1.tile([H1, PSUM_COLS], F32)
                    nc.tensor.matmul(
                        p1[:, 0:ncol], lg, kv[:, j0 : j0 + nb, 0:cg],
                        start=True, stop=False,
                    )
                    nc.tensor.matmul(
                        p1[:, 0:ncol], sb_P, qv[:, j0 : j0 + nb, 0:cg],
                        start=False, stop=True,
                    )
                    st_[i] = {"p1": p1}

                def stB(i):
                    gb, nb, cg = plist[i]
                    ncol = nb * cg
                    h1_t = h1p.tile([H1, PSUM_COLS], BF16, tag="h1_t")
                    nc.scalar.activation(
                        out=h1_t[:, 0:ncol],
                        in_=st_[i]["p1"][:, 0:ncol],
                        func=mybir.ActivationFunctionType.Sigmoid,
                    )
                    st_[i]["h1"] = h1_t

                def stC(i):
                    gb, nb, cg = plist[i]
                    ncol = nb * cg
                    p2 = ps2.tile([H2, PSUM_COLS], F32)
                    nc.tensor.matmul(
                        p2[:, 0:ncol], sb_W2, st_[i]["h1"][:, 0:ncol],
                        start=True, stop=True,
                    )
                    st_[i]["p2"] = p2

                def stD(i):
                    gb, nb, cg = plist[i]
                    ncol = nb * cg
                    h2_t = h2p.tile([H2, PSUM_COLS], BF16, tag="h2_t")
                    nc.scalar.activation(
                        out=h2_t[:, 0:ncol],
                        in_=st_[i]["p2"][:, 0:ncol],
                        func=mybir.ActivationFunctionType.Sigmoid,
                        bias=sb_b2[:, 0:1],
                    )
                    st_[i]["h2"] = h2_t

                def stE(i):
                    gb, nb, cg = plist[i]
                    ncol = nb * cg
                    p3 = ps3.tile([2, PSUM_COLS], F32)
                    nc.tensor.matmul(
                        p3[:, 0:ncol], sb_W3r, st_[i]["h2"][:, 0:ncol],
                        start=True, stop=True,
                    )
                    st_[i]["p3"] = p3

                def stF(i):
                    gb, nb, cg = plist[i]
                    g4 = gb // 32
                    sa = sc_blk[g4][:]
                    p3t = st_[i]["p3"][:]
                    nc.vector.tensor_copy(
                        out=bass.AP(
                            tensor=sa.tensor,
                            offset=sa.offset + (gb - g4 * 32) * T,
                            ap=[sa.ap[0], [T, nb], [1, cg]],
                        ),
                        in_=bass.AP(
                            tensor=p3t.tensor,
                            offset=p3t.offset,
                            ap=[p3t.ap[0], [cg, nb], [1, cg]],
                        ),
                    )
                    st_[i] = None
                    if blk_last[g4] == i:
                        # relayout 32 batches of scores into the strip
                        st = strip[:]
                        for r in range(2):
                            nc.sync.dma_start(
                                out=bass.AP(
                                    tensor=st.tensor,
                                    offset=st.offset
                                    + (g4 * 32 + r) * st.ap[0][0],
                                    ap=[[2 * st.ap[0][0], 16], [1, T]],
                                ),
                                in_=bass.AP(
                                    tensor=sa.tensor,
                                    offset=sa.offset + r * sa.ap[0][0] + r * T,
                                    ap=[[sa.ap[0][0], 1], [2 * T, 16], [1, T]],
                                ),
                            )
                        del sc_blk[g4]

                # 2-stage software pipeline: PE gets A(i), C(i-1), E(i-2);
                # ACT gets B(i), D(i-1); DVE gets F(i-2).  Engine queues
                # are in-order, so the skew keeps every queue head's
                # dependencies already satisfied.
                for i in range(n + 2):
                    if i < n:
                        stA(i)
                    if i >= 1 and i - 1 < n:
                        stC(i - 1)
                    if i >= 2:
                        stE(i - 2)
                    if i < n:
                        stB(i)
                    if i >= 1 and i - 1 < n:
                        stD(i - 1)
                    if i >= 2:
                        stF(i - 2)

                # ---- softmax over t for 128 batches ----
                nc.vector.tensor_tensor(
                    out=strip,
                    in0=strip,
                    in1=sb_mask[:, s * T : (s + 1) * T],
                    op=mybir.AluOpType.add,
                )
                ew = softp.tile([128, T], BF16)
                esum = softp.tile([128, 1], F32)
                nc.scalar.activation(
                    out=ew,
                    in_=strip,
                    func=mybir.ActivationFunctionType.Exp,
                    accum_out=esum,
                )
                rsum = softp.tile([128, 1], F32)
                nc.vector.reciprocal(out=rsum, in_=esum)

                # ---- weighted sum: out[b, e] = sum_t w[b,t] * keys[b,t,e]
                TH = T // 2
                tcs = tcs_list[s]
                o_h = []
                for h in range(2):
                    tc_h = min(TH, max(0, tcs - h * TH))
                    if tc_h == 0:
                        continue
                    kn_t = knp.tile([128, TH * E], BF16, tag="kn_t")
                    nc.sync.dma_start(
                        out=kn_t[:, 0 : tc_h * E],
                        in_=knat[
                            s * 128 : (s + 1) * 128,
                            h * TH * E : (h * TH + tc_h) * E,
                        ],
                    )
                    ewap = ew[:]
                    w_b = bass.AP(
                        tensor=ewap.tensor,
                        offset=ewap.offset + h * TH,
                        ap=[ewap.ap[0], [1, tc_h], [0, E]],
                    )
                    kview = kn_t.rearrange("b (t e) -> b t e", e=E)[:, 0:tc_h, :]
                    nc.gpsimd.tensor_tensor(
                        out=kview, in0=kview, in1=w_b, op=mybir.AluOpType.mult
                    )
                    o_t = outp.tile([128, E], F32, tag=f"oh{h}")
                    nc.vector.tensor_reduce(
                        out=o_t,
                        in_=bass.AP(
                            tensor=kn_t.tensor,
                            offset=kn_t[:].offset,
                            ap=[kn_t[:].ap[0], [1, E], [E, tc_h]],
                        ),
                        axis=mybir.AxisListType.X,
                        op=mybir.AluOpType.add,
                    )
                    o_h.append(o_t)
                if len(o_h) == 2:
                    o_s = outp.tile([128, E], F32, tag="os")
                    nc.vector.tensor_add(out=o_s, in0=o_h[0], in1=o_h[1])
                else:
                    o_s = o_h[0]
                o_f = outp.tile([128, E], F32, tag="of")
                rs = rsum[:]
                nc.vector.tensor_tensor(
                    out=o_f,
                    in0=o_s,
                    in1=bass.AP(tensor=rs.tensor, offset=rs.offset,
                                ap=[rs.ap[0], [0, E]]),
                    op=mybir.AluOpType.mult,
                )
                nc.sync.dma_start(out=out[s * 128 : (s + 1) * 128, :], in_=o_f)

    return nc


_SEQ_OK = {"EventSemaphore", "ISA", "RegisterMove", "RegisterAluOp"}


def _legalize_waits(bir_bytes):
    """This container's walrus rejects compute instructions that carry a
    DMA-semaphore wait alongside any other wait ("Too many sync wait
    commands").  Move every DMA-sem wait of a multi-wait compute
    instruction onto its own same-engine EventSemaphore (pure sequencer
    wait) inserted right before it -- semantics are identical, the
    sequencer simply performs the waits one instruction earlier."""
    d = json.loads(bir_bytes)
    for fn in d["functions"]:
        for bb in fn["blocks"]:
            out = []
            for ins in bb["instructions"]:
                si = ins.get("sync_info")
                waits = (si or {}).get("on_wait") or []
                if si and len(waits) >= 2 and ins.get("opcode") not in _SEQ_OK:
                    eng = [
                        w
                        for w in waits
                        if not str(w.get("ant_name", "")).startswith("DMA")
                    ]
                    kept = eng[-1] if eng else waits[-1]
                    moved = [w for w in waits if w is not kept]
                    for k, w in enumerate(moved):
                        out.append(
                            {
                                "name": f"{ins['name']}_lw{k}",
                                "opcode": "EventSemaphore",
                                "engine": ins["engine"],
                                "debug": ins.get("debug", 0),
                                "ins": [],
                                "outs": [],
                                "sync_info": {
                                    "on_wait": [w],
                                    "on_update": [],
                                },
                            }
                        )
                    si["on_wait"] = [kept]
                out.append(ins)
            bb["instructions"] = out
    return json.dumps(d).encode()


import ml_dtypes

BF16_NP = np.dtype(ml_dtypes.bfloat16)


def _prep_wall(W1, b1, W2, b2, W3):
    W1 = np.asarray(W1, np.float32)
    W1q, W1k, W1d, W1p = W1[0:64], W1[64:128], W1[128:192], W1[192:256]
    wall = np.zeros((128, 1492), np.float32)
    wall[0:64, 0:80] = W1k - W1d          # A
    wall[0:64, 80:160] = W1p              # P
    wall[0:64, 160:240] = W1q + W1d       # Wqd
    wall[64:128, 0:80] = wall[0:64, 0:80]
    wall[64:128, 80:160] = wall[0:64, 80:160]
    wall[0:80, 240:280] = np.asarray(W2, np.float32)
    wall[0:40, 280:282] = np.repeat(np.asarray(W3, np.float32), 2, axis=1)
    wall[0:40, 282] = np.asarray(b2, np.float32)
    wall[:, 284:364] = np.asarray(b1, np.float32)[None, :]
    wall[:, 364:492] = np.eye(128, dtype=np.float32)
    return wall


def kernel(query, keys, keys_length, W1, b1, W2, b2, W3, b3, _trace=False):
    query = np.asarray(query, np.float32)
    keys = np.asarray(keys, np.float32)
    lens = np.asarray(keys_length).reshape(4096, 1)

    wall_w = _prep_wall(W1, b1, W2, b2, W3)

    # sort each core's batches by length (desc); compute caps are the
    # per-slot max across cores, so one SPMD program serves all 8 cores
    orders = [
        np.argsort(-lens[c * BC : (c + 1) * BC, 0], kind="stable")
        for c in range(NCORES)
    ]
    sorted_lens = np.stack(
        [lens[c * BC : (c + 1) * BC, 0][orders[c]] for c in range(NCORES)]
    )
    caps = np.clip(
        (np.max(sorted_lens, axis=0).astype(np.int64) + 7) // 8 * 8, 8, T
    )
    caps = [int(x) for x in caps]
    # weighted-sum truncation: supertile max cap (len-0 batches are
    # handled host-side below)
    tcs_list = [int(caps[s * 128]) for s in range(NSUPER)]
    packs = _make_packs(caps)
    nc = build_nc(packs, tcs_list)
    patched = _legalize_waits(nc.to_json_bytes())
    nc.to_json_bytes = lambda: patched

    in_maps = []
    for c in range(NCORES):
        od = orders[c]
        kc = keys[c * BC : (c + 1) * BC][od]                  # [BC, T, E]
        qc = query[c * BC : (c + 1) * BC, 0, :][od]           # [BC, E]
        lc = lens[c * BC : (c + 1) * BC, 0][od].astype(np.int64)
        tt = np.arange(T)[None, :]
        mc = np.where(tt < lc[:, None], 0.0, MASK_NEG).astype(np.float32)
        # [BC, T] -> [128, NSUPER*T]: column-block s holds supertile s
        mc = np.ascontiguousarray(
            mc.reshape(NSUPER, 128, T).transpose(1, 0, 2).reshape(128, NSUPER * T)
        )
        kcb = kc.astype(BF16_NP)
        # kT2 [64, BC*T]: plain e-on-partition transposed keys, batch-major
        kT2 = np.ascontiguousarray(
            kcb.transpose(2, 0, 1).reshape(E, BC * T)
        )
        qcT = qc.T                               # [64, BC]
        wqc = np.zeros((128, 1804 + 16 * T), np.float32)
        wqc[:, 0:492] = wall_w[:, 0:492]
        wqc[:, 492 : 492 + NSUPER * T] = mc
        wqc[0:E, 1292 : 1292 + BC] = qcT
        # one-hot selector rows (partitions 64:80): row j = 1 on batch j's
        # 200 t-columns of a 16-batch keys tile
        for j in range(16):
            wqc[E + j, 1804 + j * T : 1804 + (j + 1) * T] = 1.0
        in_maps.append(
            {
                "kT2": kT2,
                "knat": np.ascontiguousarray(kcb.reshape(BC, T * E)),
                "wall": np.ascontiguousarray(wqc.astype(BF16_NP)),
            }
        )

    res = run_bass_kernel_spmd(nc, in_maps, core_ids=list(range(NCORES)),
                               trace=_trace)
    outs = []
    for c in range(NCORES):
        blk = np.empty((BC, E), np.float32)
        blk[orders[c]] = res.results[c]["out"]
        # length-0 batches: reference softmax is uniform -> mean of keys
        lc = lens[c * BC : (c + 1) * BC, 0]
        z = np.nonzero(lc == 0)[0]
        if len(z):
            blk[z] = keys[c * BC : (c + 1) * BC][z].mean(axis=1)
        outs.append(blk)
    full = np.concatenate(outs, axis=0)[:, None, :]
    if _trace:
        kernel._last_exec_ns = res.exec_time_ns
        kernel._last_results = res
    return full.astype(np.float32)


# revision 16
# speedup vs baseline: 1.0383x; 1.0100x over previous
"""AttentionSequencePoolingLayer (DIN-style) Trainium2 Bass kernel.

Math (per batch b, position t):
  att_in = [q, k, q-k, q*k] @ W1 + b1
         = k @ A + (q*k) @ P + (q @ (W1q+W1d) + b1)     [algebraic refactor]
    where W1 = [W1q; W1k; W1d; W1p], A = W1k - W1d, P = W1p.
  h1 = sigmoid(...); h2 = sigmoid(h1 @ W2 + b2); score = h2 @ W3 (+ b3,
  dropped: softmax is shift-invariant).  Masked positions get -80 (exp ~ 0).
  out[b] = softmax(score) @ keys[b].

Layout strategy (per core, 512 batches, all on-chip data bf16):
  - batches sorted by length desc (host); per-slot compute caps = max len
    across cores (one SPMD program serves all 8 cores).  All MLP matmuls,
    sigmoids and score copies are truncated to the cap.
  - kT2 HBM layout [128, BC*T/2]: partition p=(hi,e) holds keys feature e
    for 16-batch half hi of each 32-batch group; every keys DMA is a plain
    [128, N] copy whose descriptors spread across all 16 DMA engines (a
    2-outer-dim AP confines a DMA to 2 engines).
  - MLP runs in transposed layout (features on partitions, (b,t) on free
    dim), batches PACKED into <=512-column PSUM tiles: nb consecutive
    batches share one tile at the pack's max cap, so sigmoid/copies are
    dense (no per-batch slicing) and per-matmul dispatch is amortized.
  - per-batch layer-1 bias (aT = q@(W1q+W1d)+b1, one matmul per supertile
    at setup) enters through a K=128 identity-selector matmul.
  - scores land in [2, nb*cg] PSUM, are copied (DVE) into a [2, 32*200]
    bf16 staging row at 200/batch stride, then 2 strided SBUF->SBUF DMAs
    per 32 batches relayout to a [128b, 200t] strip for softmax (ACT exp
    with fused f32 sum).
  - weighted sum from natural-layout keys (knat, bf16, truncated to the
    supertile cap): multiply on GPSIMD (idle engine), segmented t-reduce
    on DVE (f32 accum), then combine + 1/sum normalize.
  - length-0 batches (softmax over all-masked = uniform mean of keys) are
    computed host-side and overwrite those rows; the device path may
    produce garbage for them.

Compiler workaround: this container's walrus rejects instructions with
more than one semaphore wait; _legalize_waits() rewrites the BIR so every
excess wait rides its own same-engine EventSemaphore.
"""

import json
import sys

import numpy as np

try:
    import concourse.bass as bass
except ImportError:
    sys.path.insert(0, "/opt/trn_rl_repo")
    import concourse.bass as bass
import concourse.mybir as mybir
import concourse.tile as tile
from concourse.bass_utils import run_bass_kernel_spmd

E = 64
T = 200
H1, H2 = 80, 40
NCORES = 8
BC = 4096 // NCORES          # batches per core
NSUPER = BC // 128           # supertiles of 128 batches
NGRP = BC // 32              # 32-batch keys groups (one kdual DMA each)
MASK_NEG = -80.0
PSUM_COLS = 512

F32 = mybir.dt.float32
BF16 = mybir.dt.bfloat16


def _bcast(ap2d, c0, nb, nt):
    """From [P, C] SBUF ap: [P, nb, nt] AP broadcasting col c over nt."""
    base = ap2d[:, c0 : c0 + nb]
    return bass.AP(
        tensor=base.tensor,
        offset=base.offset,
        ap=[base.ap[0], base.ap[1], [0, nt]],
    )


def _make_packs(caps):
    """Per supertile: list of (slot, nb, cg) packs.  Slots are sorted by
    cap desc; packs stay inside 16-slot halves so each maps to one
    partition half of its 32-batch group."""
    packs = []
    for s in range(NSUPER):
        ps = []
        b = 0
        while b < 128:
            cg = caps[s * 128 + b]
            nb = min(PSUM_COLS // cg, 16 - b % 16)
            ps.append((b, nb, cg))
            b += nb
        packs.append(ps)
    return packs


def build_nc(packs, tcs_list):
    nc = bass.Bass("TRN2")

    kT2 = nc.dram_tensor("kT2", [E, BC * T], BF16, kind="ExternalInput")
    knat = nc.dram_tensor("knat", [BC, T * E], BF16, kind="ExternalInput")
    wall = nc.dram_tensor("wall", [128, 1804 + 16 * T], BF16, kind="ExternalInput")
    out = nc.dram_tensor("out", [BC, E], F32, kind="ExternalOutput")

    with tile.TileContext(nc) as tc:
        with (
            tc.tile_pool(name="consts", bufs=1) as consts,
            tc.tile_pool(name="ktp", bufs=3) as ktp,
            tc.tile_pool(name="qkp", bufs=3) as qkp,
            tc.tile_pool(name="lgp", bufs=3) as lgp,
            tc.tile_pool(name="h1p", bufs=3) as h1p,
            tc.tile_pool(name="h2p", bufs=3) as h2p,
            tc.tile_pool(name="scp", bufs=2) as scp,
            tc.tile_pool(name="stripp", bufs=2) as stripp,
            tc.tile_pool(name="softp", bufs=2) as softp,
            tc.tile_pool(name="knp", bufs=2) as knp,
            tc.tile_pool(name="outp", bufs=2) as outp,
            tc.tile_pool(name="ps1", bufs=3, space="PSUM") as ps1,
            tc.tile_pool(name="ps2", bufs=2, space="PSUM") as ps2,
            tc.tile_pool(name="ps3", bufs=2, space="PSUM") as ps3,
            tc.tile_pool(name="psg", bufs=1, space="PSUM") as psg,
        ):
            # ---- constants (one DMA for all weights + qT/qsh + mask) ----
            sb_wall0 = consts.tile([128, 1804 + 16 * T], BF16)
            nc.sync.dma_start(out=sb_wall0, in_=wall[:, :])
            # copy through DVE so every consumer waits on an engine sem
            sb_wall = consts.tile([128, 1804 + 16 * T], BF16)
            nc.vector.tensor_copy(out=sb_wall, in_=sb_wall0)
            sb_A = sb_wall[0:E, 0:H1]
            sb_P = sb_wall[0:E, 80:160]
            sb_Wqd = sb_wall[0:E, 160:240]
            sb_W2 = sb_wall[0:H1, 240:280]
            sb_W3r = sb_wall[0:H2, 280:282]
            sb_b2 = sb_wall[0:H2, 282:283]
            sb_b1rep16 = sb_wall[E : E + 16, 284:364]
            sb_mask = sb_wall[:, 492 : 492 + NSUPER * T]
            sb_qT = sb_wall[0:E, 1292 : 1292 + BC]          # [64, BC]
            sb_oh = sb_wall[E : E + 16, 1804 : 1804 + 16 * T]

            # seed the rotating pool slots: keys tiles get the constant
            # one-hot selector rows (64:80), lhsT tiles get A (rows 0:64);
            # later writers only touch the complementary partition rows, so
            # these persist across slot reuse
            for _ in range(3):
                kd0 = ktp.tile([E + 16, 16 * T], BF16, tag="kd")
                nc.vector.tensor_copy(out=kd0[E : E + 16, :], in_=sb_oh)
                lg0 = lgp.tile([H1, H1], BF16, tag="lg")
                nc.vector.tensor_copy(out=lg0[0:E, :], in_=sb_A)

            # staging slots hold stale data beyond each batch's cap; first
            # use must be finite (later reuse leaves bounded old scores)
            for _ in range(2):
                sc_t = scp.tile([2, 32 * T], BF16, tag="sc")
                nc.vector.memset(sc_t[:, :], MASK_NEG)

            # ---- 16-batch group prefetch: keys DMA (rows 0:64), q*k on
            # GPSIMD, and the merged lhsT [A; aT16] (bias rows via a tiny
            # base-64 matmul + DVE add; one-hot rhs rows make the bias land
            # per batch, replacing the old K=128 selector matmul) ----
            kduals = {}

            def prefetch(g):
                s_, gl_ = g // 8, g % 8
                kd = ktp.tile([E + 16, 16 * T], BF16, tag="kd")
                nc.sync.dma_start(
                    out=kd[0:E, :], in_=kT2[:, g * 16 * T : (g + 1) * 16 * T]
                )
                cgg = packs[s_][
                    [i for i, p in enumerate(packs[s_]) if p[0] == gl_ * 16][0]
                ][2]
                qk = qkp.tile([E, 16 * T], BF16, tag="qk")
                nc.gpsimd.tensor_tensor(
                    out=qk.rearrange("p (b t) -> p b t", t=T)[:, :, 0:cgg],
                    in0=kd[0:E, :].rearrange("p (b t) -> p b t", t=T)[
                        :, :, 0:cgg
                    ],
                    in1=_bcast(sb_qT, g * 16, 16, cgg),
                    op=mybir.AluOpType.mult,
                )
                aT_ps = psg.tile([128, H1], F32, tag="psg")
                nc.tensor.matmul(
                    aT_ps[E : E + 16, :],
                    sb_qT[:, g * 16 : (g + 1) * 16],
                    sb_Wqd,
                    start=True,
                    stop=True,
                )
                lg = lgp.tile([H1, H1], BF16, tag="lg")
                nc.vector.tensor_tensor(
                    out=lg[E : E + 16, :],
                    in0=aT_ps[E : E + 16, :],
                    in1=sb_b1rep16,
                    op=mybir.AluOpType.add,
                )
                kduals[g] = (kd, qk, lg)

            prefetch(0)

            for s in range(NSUPER):
                strip = stripp.tile([128, T], BF16)
                plist = packs[s]
                n = len(plist)
                blk_last = {}          # g4 -> index of last pack in block
                for i, p in enumerate(plist):
                    blk_last[p[0] // 32] = i
                st_ = {}               # pack idx -> per-stage artifacts
                sc_blk = {}            # g4 -> staging tile

                def stA(i):
                    gb, nb, cg = plist[i]
                    g4 = gb // 32
                    if gb % 16 == 0:   # first pack of a 16-batch group
                        nxt = s * 8 + gb // 16 + 1
                        if nxt < NSUPER * 8:
                            prefetch(nxt)
                    if gb % 32 == 0:   # first pack of a staging block
                        sc_t = scp.tile([2, 32 * T], BF16, tag="sc")
                        # tiny touch so the slot-reuse DMA waits land here
                        nc.vector.memset(sc_t[0:2, 0:1], MASK_NEG)
                        sc_blk[g4] = sc_t
                    kd, qk, lg = kduals[s * 8 + gb // 16]
                    j0 = gb % 16
                    ncol = nb * cg
                    kv = kd.rearrange("p (b t) -> p b t", t=T)
                    qv = qk.rearrange("p (b t) -> p b t", t=T)
                    p1 = ps1.tile([H1, PSUM_COLS], F32)
                    nc.tensor.matmul(
                        p1[:, 0:ncol], lg, kv[:, j0 : j0 + nb, 0:cg],
                        start=True, stop=False,
                    )
                    nc.tensor.matmul(
                        p1[:, 0:ncol], sb_P, qv[:, j0 : j0 + nb, 0:cg],
                        start=False, stop=True,
                    )
                    st_[i] = {"p1": p1}

                def stB(i):
                    gb, nb, cg = plist[i]
                    ncol = nb * cg
                    h1_t = h1p.tile([H1, PSUM_COLS], BF16, tag="h1_t")
                    nc.scalar.activation(
                        out=h1_t[:, 0:ncol],
                        in_=st_[i]["p1"][:, 0:ncol],
                        func=mybir.ActivationFunctionType.Sigmoid,
                    )
                    st_[i]["h1"] = h1_t

                def stC(i):
                    gb, nb, cg = plist[i]
                    ncol = nb * cg
                    p2 = ps2.tile([H2, PSUM_COLS], F32)
                    nc.tensor.matmul(
                        p2[:, 0:ncol], sb_W2, st_[i]["h1"][:, 0:ncol],
                        start=True, stop=True,
                    )
                    st_[i]["p2"] = p2

                def stD(i):
                    gb, nb, cg = plist[i]
                    ncol = nb * cg
                    h2_t = h2p.tile([H2, PSUM_COLS], BF16, tag="h2_t")
                    nc.scalar.activation(
                        out=h2_t[:, 0:ncol],
                        in_=st_[i]["p2"][:, 0:ncol],
                        func=mybir.ActivationFunctionType.Sigmoid,
                        bias=sb_b2[:, 0:1],
                    )
                    st_[i]["h2"] = h2_t

                def stE(i):
                    gb, nb, cg = plist[i]
                    ncol = nb * cg
                    p3 = ps3.tile([2, PSUM_COLS], F32)
                    nc.tensor.matmul(
                        p3[:, 0:ncol], sb_W3r, st_[i]["h2"][:, 0:ncol],
                        start=True, stop=True,
                    )
                    st_[i]["p3"] = p3

                def stF(i):
                    gb, nb, cg = plist[i]
                    g4 = gb // 32
                    sa = sc_blk[g4][:]
                    p3t = st_[i]["p3"][:]
                    nc.vector.tensor_copy(
                        out=bass.AP(
                            tensor=sa.tensor,
                            offset=sa.offset + (gb - g4 * 32) * T,
                            ap=[sa.ap[0], [T, nb], [1, cg]],
                        ),
                        in_=bass.AP(
                            tensor=p3t.tensor,
                            offset=p3t.offset,
                            ap=[p3t.ap[0], [cg, nb], [1, cg]],
                        ),
                    )
                    st_[i] = None
                    if blk_last[g4] == i:
                        # relayout 32 batches of scores into the strip
                        st = strip[:]
                        for r in range(2):
                            nc.sync.dma_start(
                                out=bass.AP(
                                    tensor=st.tensor,
                                    offset=st.offset
                                    + (g4 * 32 + r) * st.ap[0][0],
                                    ap=[[2 * st.ap[0][0], 16], [1, T]],
                                ),
                                in_=bass.AP(
                                    tensor=sa.tensor,
                                    offset=sa.offset + r * sa.ap[0][0] + r * T,
                                    ap=[[sa.ap[0][0], 1], [2 * T, 16], [1, T]],
                                ),
                            )
                        del sc_blk[g4]

                # 2-stage software pipeline: PE gets A(i), C(i-1), E(i-2);
                # ACT gets B(i), D(i-1); DVE gets F(i-2).  Engine queues
                # are in-order, so the skew keeps every queue head's
                # dependencies already satisfied.
                for i in range(n + 2):
                    if i < n:
                        stA(i)
                    if i >= 1 and i - 1 < n:
                        stC(i - 1)
                    if i >= 2:
                        stE(i - 2)
                    if i < n:
                        stB(i)
                    if i >= 1 and i - 1 < n:
                        stD(i - 1)
                    if i >= 2:
                        stF(i - 2)

                # ---- softmax over t for 128 batches ----
                nc.vector.tensor_tensor(
                    out=strip,
                    in0=strip,
                    in1=sb_mask[:, s * T : (s + 1) * T],
                    op=mybir.AluOpType.add,
                )
                ew = softp.tile([128, T], BF16)
                esum = softp.tile([128, 1], F32)
                nc.scalar.activation(
                    out=ew,
                    in_=strip,
                    func=mybir.ActivationFunctionType.Exp,
                    accum_out=esum,
                )
                rsum = softp.tile([128, 1], F32)
                nc.vector.reciprocal(out=rsum, in_=esum)

                # ---- weighted sum: out[b, e] = sum_t w[b,t] * keys[b,t,e]
                TH = T // 2
                tcs = tcs_list[s]
                o_h = []
                for h in range(2):
                    tc_h = min(TH, max(0, tcs - h * TH))
                    if tc_h == 0:
                        continue
                    kn_t = knp.tile([128, TH * E], BF16, tag="kn_t")
                    nc.sync.dma_start(
                        out=kn_t[:, 0 : tc_h * E],
                        in_=knat[
                            s * 128 : (s + 1) * 128,
                            h * TH * E : (h * TH + tc_h) * E,
                        ],
                    )
                    ewap = ew[:]
                    w_b = bass.AP(
                        tensor=ewap.tensor,
                        offset=ewap.offset + h * TH,
                        ap=[ewap.ap[0], [1, tc_h], [0, E]],
                    )
                    kview = kn_t.rearrange("b (t e) -> b t e", e=E)[:, 0:tc_h, :]
                    nc.gpsimd.tensor_tensor(
                        out=kview, in0=kview, in1=w_b, op=mybir.AluOpType.mult
                    )
                    o_t = outp.tile([128, E], F32, tag=f"oh{h}")
                    nc.vector.tensor_reduce(
                        out=o_t,
                        in_=bass.AP(
                            tensor=kn_t.tensor,
                            offset=kn_t[:].offset,
                            ap=[kn_t[:].ap[0], [1, E], [E, tc_h]],
                        ),
                        axis=mybir.AxisListType.X,
                        op=mybir.AluOpType.add,
                    )
                    o_h.append(o_t)
                if len(o_h) == 2:
                    o_s = outp.tile([128, E], F32, tag="os")
                    nc.vector.tensor_add(out=o_s, in0=o_h[0], in1=o_h[1])
                else:
                    o_s = o_h[0]
                o_f = outp.tile([128, E], F32, tag="of")
                rs = rsum[:]
                nc.vector.tensor_tensor(
                    out=o_f,
                    in0=o_s,
                    in1=bass.AP(tensor=rs.tensor, offset=rs.offset,
                                ap=[rs.ap[0], [0, E]]),
                    op=mybir.AluOpType.mult,
                )
                nc.sync.dma_start(out=out[s * 128 : (s + 1) * 128, :], in_=o_f)

    return nc


_SEQ_OK = {"EventSemaphore", "ISA", "RegisterMove", "RegisterAluOp"}


def _legalize_waits(bir_bytes):
    """This container's walrus rejects compute instructions that carry a
    DMA-semaphore wait alongside any other wait ("Too many sync wait
    commands").  Move every DMA-sem wait of a multi-wait compute
    instruction onto its own same-engine EventSemaphore (pure sequencer
    wait) inserted right before it -- semantics are identical, the
    sequencer simply performs the waits one instruction earlier."""
    d = json.loads(bir_bytes)
    for fn in d["functions"]:
        for bb in fn["blocks"]:
            out = []
            for ins in bb["instructions"]:
                si = ins.get("sync_info")
                waits = (si or {}).get("on_wait") or []
                if si and len(waits) >= 2 and ins.get("opcode") not in _SEQ_OK:
                    eng = [
                        w
                        for w in waits
                        if not str(w.get("ant_name", "")).startswith("DMA")
                    ]
                    kept = eng[-1] if eng else waits[-1]
                    moved = [w for w in waits if w is not kept]
                    for k, w in enumerate(moved):
                        out.append(
                            {
                                "name": f"{ins['name']}_lw{k}",
                                "opcode": "EventSemaphore",
                                "engine": ins["engine"],
                                "debug": ins.get("debug", 0),
                                "ins": [],
                                "outs": [],
                                "sync_info": {
                                    "on_wait": [w],
                                    "on_update": [],
                                },
                            }
                        )
                    si["on_wait"] = [kept]
                out.append(ins)
            bb["instructions"] = out
    return json.dumps(d).encode()


import ml_dtypes

BF16_NP = np.dtype(ml_dtypes.bfloat16)


def _prep_wall(W1, b1, W2, b2, W3):
    W1 = np.asarray(W1, np.float32)
    W1q, W1k, W1d, W1p = W1[0:64], W1[64:128], W1[128:192], W1[192:256]
    wall = np.zeros((128, 1492), np.float32)
    wall[0:64, 0:80] = W1k - W1d          # A
    wall[0:64, 80:160] = W1p              # P
    wall[0:64, 160:240] = W1q + W1d       # Wqd
    wall[64:128, 0:80] = wall[0:64, 0:80]
    wall[64:128, 80:160] = wall[0:64, 80:160]
    wall[0:80, 240:280] = np.asarray(W2, np.float32)
    wall[0:40, 280:282] = np.repeat(np.asarray(W3, np.float32), 2, axis=1)
    wall[0:40, 282] = np.asarray(b2, np.float32)
    wall[:, 284:364] = np.asarray(b1, np.float32)[None, :]
    wall[:, 364:492] = np.eye(128, dtype=np.float32)
    return wall


def kernel(query, keys, keys_length, W1, b1, W2, b2, W3, b3, _trace=False):
    query = np.asarray(query, np.float32)
    keys = np.asarray(keys, np.float32)
    lens = np.asarray(keys_length).reshape(4096, 1)

    wall_w = _prep_wall(W1, b1, W2, b2, W3)

    # sort each core's batches by length (desc); compute caps are the
    # per-slot max across cores, so one SPMD program serves all 8 cores
    orders = [
        np.argsort(-lens[c * BC : (c + 1) * BC, 0], kind="stable")
        for c in range(NCORES)
    ]
    sorted_lens = np.stack(
        [lens[c * BC : (c + 1) * BC, 0][orders[c]] for c in range(NCORES)]
    )
    caps = np.clip(
        (np.max(sorted_lens, axis=0).astype(np.int64) + 7) // 8 * 8, 8, T
    )
    caps = [int(x) for x in caps]
    # weighted-sum truncation: supertile max cap (len-0 batches are
    # handled host-side below)
    tcs_list = [int(caps[s * 128]) for s in range(NSUPER)]
    packs = _make_packs(caps)
    nc = build_nc(packs, tcs_list)
    patched = _legalize_waits(nc.to_json_bytes())
    nc.to_json_bytes = lambda: patched

    in_maps = []
    for c in range(NCORES):
        od = orders[c]
        kc = keys[c * BC : (c + 1) * BC][od]                  # [BC, T, E]
        qc = query[c * BC : (c + 1) * BC, 0, :][od]           # [BC, E]
        lc = lens[c * BC : (c + 1) * BC, 0][od].astype(np.int64)
        tt = np.arange(T)[None, :]
        mc = np.where(tt < lc[:, None], 0.0, MASK_NEG).astype(np.float32)
        # [BC, T] -> [128, NSUPER*T]: column-block s holds supertile s
        mc = np.ascontiguousarray(
            mc.reshape(NSUPER, 128, T).transpose(1, 0, 2).reshape(128, NSUPER * T)
        )
        kcb = kc.astype(BF16_NP)
        # kT2 [64, BC*T]: plain e-on-partition transposed keys, batch-major
        kT2 = np.ascontiguousarray(
            kcb.transpose(2, 0, 1).reshape(E, BC * T)
        )
        qcT = qc.T                               # [64, BC]
        wqc = np.zeros((128, 1804 + 16 * T), np.float32)
        wqc[:, 0:492] = wall_w[:, 0:492]
        wqc[:, 492 : 492 + NSUPER * T] = mc
        wqc[0:E, 1292 : 1292 + BC] = qcT
        # one-hot selector rows (partitions 64:80): row j = 1 on batch j's
        # 200 t-columns of a 16-batch keys tile
        for j in range(16):
            wqc[E + j, 1804 + j * T : 1804 + (j + 1) * T] = 1.0
        in_maps.append(
            {
                "kT2": kT2,
                "knat": np.ascontiguousarray(kcb.reshape(BC, T * E)),
                "wall": np.ascontiguousarray(wqc.astype(BF16_NP)),
            }
        )

    res = run_bass_kernel_spmd(nc, in_maps, core_ids=list(range(NCORES)),
                               trace=_trace)
    outs = []
    for c in range(NCORES):
        blk = np.empty((BC, E), np.float32)
        blk[orders[c]] = res.results[c]["out"]
        # length-0 batches: reference softmax is uniform -> mean of keys
        lc = lens[c * BC : (c + 1) * BC, 0]
        z = np.nonzero(lc == 0)[0]
        if len(z):
            blk[z] = keys[c * BC : (c + 1) * BC][z].mean(axis=1)
        outs.append(blk)
    full = np.concatenate(outs, axis=0)[:, None, :]
    if _trace:
        kernel._last_exec_ns = res.exec_time_ns
        kernel._last_results = res
    return full.astype(np.float32)
